# revision 25
# baseline (speedup 1.0000x reference)
"""MoE ExpertCombiner (scatter-add) Trainium2 Bass kernel.

  out[b, s, :] = sum over (e, c) with token_indices[e,c] == b*S+s of
                 weights[e, c] * expert_outputs[e, c, :]

Strategy (8 NeuronCores, SPMD):
  Host: flatten the (e, c) rows, stable-sort by destination token, and
  shard the TOKEN space contiguously across the 8 cores (each core owns
  4096 destination tokens and receives exactly the sorted rows that land
  in its range -> no cross-core reduction at all; outputs concatenate).

  Device: the scatter-add becomes block-diagonal one-hot matmuls.  For
  each 128-token output window, PSUM accumulates
      onehot[rows_chunk, 128].T @ x[rows_chunk, D]
  over the few 128-row chunks of the sorted stream that overlap the
  window.  The per-row combine weight is folded into the one-hot on
  VectorE ((iota == idx) * w in a single tensor_scalar), so the PE does
  the weighting for free and no per-element multiply over D is needed.

  Everything bulky moves as bf16: rows are pre-cast on the host (the
  2e-2 harness tolerance leaves ~8x headroom over bf16's ~2.5e-3), and
  the f32 PSUM result is cast to bf16 on the PSUM->SBUF copy.  The
  core-local output lives in DRAM as [128, n_win*D] (partition-major)
  so completed windows drain as a few 2MB DMAs; the host undoes the
  transpose when assembling the full [B,S,D] f32 output.

Per-core traffic is ~17MB in + 8MB out, close to the 358 GB/s per-core
HBM roofline for this op.
"""

import math

import numpy as np
import ml_dtypes

import concourse.bacc as bacc
import concourse.mybir as mybir
import concourse.tile as tile
from concourse import bass_utils

P = 128
F32 = mybir.dt.float32
BF16 = mybir.dt.bfloat16
NP_BF16 = ml_dtypes.bfloat16

N_CORES = 8
W_TOK = 128


def _make_plan(idx_flat, n_tokens, n_cores, w_tok=128, group_chunks=8):
    """Sort/shard/window planning. Returns plan dict (shared across cores).

    Empty-token skip: each core's owned tokens are rank-compacted to the
    non-empty ones (the harness output buffer is pre-zeroed, so tokens
    that receive no rows need no device output at all).  Device windows
    cover RANK space; the host scatters ranks back to token ids.
    """
    order = np.argsort(idx_flat, kind="stable")
    idx_s = idx_flat[order]
    tok_per_core = n_tokens // n_cores
    bounds = np.searchsorted(idx_s, np.arange(n_cores + 1) * tok_per_core)
    counts = np.diff(bounds)
    R = int(counts.max())
    nchunk = math.ceil(R / P)
    ngrp = math.ceil(nchunk / group_chunks)
    nchunk_pad = ngrp * group_chunks
    npad = nchunk_pad * P

    cnt = np.bincount(idx_flat, minlength=n_tokens)
    tok_ne = []          # per core: global token ids of its non-empty tokens
    ranks = []           # per core: rank of each of its (sorted) rows
    for m in range(n_cores):
        lo = m * tok_per_core
        t_ne = np.flatnonzero(cnt[lo:lo + tok_per_core] > 0)
        rank_of = np.full(tok_per_core, -1, np.int64)
        rank_of[t_ne] = np.arange(len(t_ne))
        tok_ne.append(t_ne + lo)
        ranks.append(rank_of[idx_s[bounds[m]:bounds[m + 1]] - lo])
    n_win = max(math.ceil(len(t) / w_tok) for t in tok_ne)

    c_lo = np.full(n_win, 1 << 30, np.int64)
    c_hi = np.full(n_win, -1, np.int64)
    for m in range(n_cores):
        il = ranks[m]
        ws = np.searchsorted(il, np.arange(n_win + 1) * w_tok)
        s_, e_ = ws[:-1], ws[1:]
        ne = e_ > s_
        # non-empty windows: chunks containing their first/last row
        c_lo[ne] = np.minimum(c_lo[ne], s_[ne] // P)
        c_hi[ne] = np.maximum(c_hi[ne], (e_[ne] - 1) // P)
        # empty windows: point at the adjacent chunk (its indices fall
        # outside the window, so the one-hot is all-zero and the matmul
        # just zeroes PSUM) without distorting any chunk's window span
        emp = ~ne
        adj = np.minimum(s_[emp] // P, nchunk - 1)
        c_lo[emp] = np.minimum(c_lo[emp], adj)
        c_hi[emp] = np.maximum(c_hi[emp], adj)
    c_lo = np.clip(c_lo, 0, nchunk - 1)
    c_hi = np.clip(c_hi, 0, nchunk - 1)
    c_hi = np.maximum(c_hi, c_lo)

    pairs = []
    win_pair_slices = []
    for w in range(n_win):
        s = len(pairs)
        for c in range(int(c_lo[w]), int(c_hi[w]) + 1):
            pairs.append((w, c))
        win_pair_slices.append((s, len(pairs)))

    chunk_wfirst = {}
    chunk_span = {}
    for w, c in pairs:
        if c not in chunk_wfirst:
            chunk_wfirst[c] = w
        chunk_span[c] = w - chunk_wfirst[c] + 1
    w_span = max(chunk_span.values()) if chunk_span else 1

    return dict(
        order=order, idx_s=idx_s, bounds=bounds, n_win=n_win, w_tok=w_tok,
        tok_per_core=tok_per_core, nchunk=nchunk, nchunk_pad=nchunk_pad,
        npad=npad, ngrp=ngrp, pairs=pairs,
        win_pair_slices=win_pair_slices, n_cores=n_cores,
        group_chunks=group_chunks, chunk_wfirst=chunk_wfirst,
        chunk_span=chunk_span, w_span=w_span,
        tok_ne=tok_ne, ranks=ranks,
    )


def _pack_core_inputs(plan, m, x_flat, w_flat, D):
    """Build in_map arrays for core m.

    rows: bf16 [P, nchunk_pad*D], layout [p, (g, k, d)] so that group g's
      chunks are contiguous columns (one big DMA per group).
    meta: [128, nchunk*2] f32
      cols [0, nchunk)          : per-chunk weight column
      cols [nchunk, 2*nchunk)   : per-chunk window-relative index column
    """
    order, idx_s, bounds = plan["order"], plan["idx_s"], plan["bounds"]
    npad, nchunk = plan["npad"], plan["nchunk"]
    w_tok, tok_per_core = plan["w_tok"], plan["tok_per_core"]
    gch, ngrp = plan["group_chunks"], plan["ngrp"]
    sel = order[bounds[m]:bounds[m + 1]]
    Rm = len(sel)
    rows = np.zeros((npad, D), NP_BF16)
    rows[:Rm] = x_flat[sel].astype(NP_BF16)
    rows = np.ascontiguousarray(
        rows.reshape(ngrp, gch, P, D).transpose(2, 0, 1, 3)
    ).reshape(P, ngrp * gch * D)

    wv = np.zeros(nchunk * P, np.float32)
    wv[:Rm] = w_flat[sel]
    il = np.full(nchunk * P, -(1 << 20), np.float32)
    il[:Rm] = plan["ranks"][m].astype(np.float32)

    meta = np.zeros((P, nchunk * 2), np.float32)
    meta[:, :nchunk] = wv.reshape(nchunk, P).T
    ilm = il.reshape(nchunk, P).T.copy()
    for c, wf in plan["chunk_wfirst"].items():
        ilm[:, c] -= wf * w_tok
    meta[:, nchunk:] = ilm

    wide = plan["w_span"] * w_tok
    iota = np.broadcast_to(np.arange(wide, dtype=np.float32), (P, wide)).copy()
    return {"rows": rows, "meta": meta, "iota": iota}


def _build_program(plan, D, n_cores, group_bufs=5, stage_bufs=3,
                   psum_bufs=4, onehot_bufs=12, kb=4, vec_cols=160,
                   oh_ahead=3):
    n_win, w_tok = plan["n_win"], plan["w_tok"]
    nchunk, nchunk_pad = plan["nchunk"], plan["nchunk_pad"]
    pairs, win_pair_slices = plan["pairs"], plan["win_pair_slices"]
    gch, ngrp = plan["group_chunks"], plan["ngrp"]
    chunk_wfirst = plan["chunk_wfirst"]
    chunk_span = plan["chunk_span"]
    w_span = plan["w_span"]
    half = min(D, 512)
    n_half = D // half
    eq = mybir.AluOpType.is_equal

    nc = bacc.Bacc("TRN2", target_bir_lowering=False, debug=False,
                   enable_asserts=False, num_devices=n_cores)
    rows_d = nc.dram_tensor("rows", [P, nchunk_pad * D], BF16,
                            kind="ExternalInput").ap()
    meta_d = nc.dram_tensor("meta", [P, nchunk * 2], F32,
                            kind="ExternalInput").ap()
    iota_d = nc.dram_tensor("iota", [P, w_span * w_tok], F32,
                            kind="ExternalInput").ap()
    out_d = nc.dram_tensor("out", [P, n_win * D], BF16,
                           kind="ExternalOutput").ap()

    with tile.TileContext(nc) as tc:
        with (
            tc.tile_pool(name="grp", bufs=group_bufs) as gpool,
            tc.tile_pool(name="misc", bufs=1) as mpool,
            tc.tile_pool(name="stage", bufs=stage_bufs) as spool,
            tc.tile_pool(name="oh", bufs=onehot_bufs) as opool,
            tc.tile_pool(name="ps", bufs=psum_bufs, space="PSUM") as ppool,
        ):
            # iota/meta lead the sync (HWDGE) queue: they're tiny (~0.26MB)
            # and the first one-hot build needs them; the SWDGE/gpsimd
            # alternative costs ~3.5us in Q7 descriptor drain
            iota_t = mpool.tile([P, w_span * w_tok], F32)
            nc.sync.dma_start(out=iota_t[:], in_=iota_d[:])
            meta_t = mpool.tile([P, nchunk * 2], F32)
            nc.sync.dma_start(out=meta_t[:], in_=meta_d[:])

            group_tiles = {}
            oh_tiles = {}

            def get_group(g):
                t = group_tiles.get(g)
                if t is None:
                    t = gpool.tile([P, gch * D], BF16, tag="grp")
                    base = g * gch * D
                    ncol = (min(nchunk, (g + 1) * gch) - g * gch) * D
                    if g < 2:
                        # per-chunk sub-DMAs so early matmuls depend on
                        # individual chunks, not the whole 2MB group; the
                        # very first chunk lands in two half-chunk pieces,
                        # then iota/meta slot in right behind it
                        for j in range(ncol // D):
                            sub = 2 if (g == 0 and j == 0) else 1
                            for q in range(sub):
                                c0 = j * D + q * D // sub
                                c1 = j * D + (q + 1) * D // sub
                                nc.sync.dma_start(
                                    out=t[:, c0:c1],
                                    in_=rows_d[:, base + c0:base + c1],
                                )
                    else:
                        nc.sync.dma_start(
                            out=t[:, :ncol],
                            in_=rows_d[:, base:base + ncol],
                        )
                    group_tiles[g] = t
                return t

            def get_oh(c):
                """One-hot (scaled by the combine weight) for chunk c over
                its window span: oh[p, t] = (il[p] == t) * w[p]."""
                t = oh_tiles.get(c)
                if t is None:
                    ncols = chunk_span.get(c, 1) * w_tok
                    t = opool.tile([P, w_span * w_tok], BF16, tag="oh")
                    nc.vector.tensor_scalar(
                        t[:, :ncols], iota_t[:, :ncols],
                        meta_t[:, nchunk + c:nchunk + c + 1],
                        meta_t[:, c:c + 1],
                        op0=eq, op1=mybir.AluOpType.mult,
                    )
                    oh_tiles[c] = t
                return t

            st = None
            for w in range(n_win):
                # prefetch one-hot builds a couple of windows ahead so the
                # PE never waits on VectorE mid-stream
                for ww in range(w, min(w + 1 + oh_ahead, n_win)):
                    ss, ee = win_pair_slices[ww]
                    for j in range(ss, ee):
                        get_oh(pairs[j][1])
                # h-outer matmul order: the low 512-col half's accumulation
                # finishes before the high half starts, so its PSUM->SBUF
                # copy overlaps the remaining matmuls (range-level deps)
                ps = ppool.tile([P, D], F32)
                s, e = win_pair_slices[w]
                for h in range(n_half):
                    hs = slice(h * half, (h + 1) * half)
                    for j in range(s, e):
                        _, c = pairs[j]
                        first, last = (j == s), (j == e - 1)
                        oh = get_oh(c)
                        g, k = divmod(c, gch)
                        gt = get_group(g)
                        off = (w - chunk_wfirst[c]) * w_tok
                        ohs = oh[:, off:off + w_tok]
                        nc.tensor.matmul(
                            ps[:, hs], ohs,
                            gt[:, k * D + h * half:k * D + (h + 1) * half],
                            start=first, stop=last,
                        )
                if w % kb == 0:
                    st = spool.tile([P, kb * D], BF16, tag="st")
                base = (w % kb) * D
                vc = D // 2 if w == n_win - 1 else vec_cols
                nc.vector.tensor_copy(st[:, base:base + vc],
                                      ps[:, :vc])
                if vc < half:
                    nc.scalar.activation(st[:, base + vc:base + half],
                                         ps[:, vc:half],
                                         mybir.ActivationFunctionType.Copy)
                nc.scalar.activation(st[:, base + half:base + D],
                                     ps[:, half:],
                                     mybir.ActivationFunctionType.Copy)
                last_stage = w >= (n_win - 1) // kb * kb
                if last_stage:
                    # tail: drain window-by-window so the final DMA is small
                    nc.scalar.dma_start(
                        out=out_d[:, w * D:(w + 1) * D],
                        in_=st[:, base:base + D],
                    )
                elif w % kb == kb - 1:
                    w0 = (w // kb) * kb
                    nc.scalar.dma_start(
                        out=out_d[:, w0 * D:(w + 1) * D],
                        in_=st[:, :kb * D],
                    )

    nc.compile()
    return nc


def kernel(expert_outputs, weights, token_indices, batch_size, seq_len):
    expert_outputs = np.ascontiguousarray(expert_outputs, dtype=np.float32)
    weights = np.ascontiguousarray(weights, dtype=np.float32)
    B, S = int(batch_size), int(seq_len)
    E, C, D = expert_outputs.shape
    n_tokens = B * S

    x_flat = expert_outputs.reshape(-1, D)
    w_flat = weights.reshape(-1)
    idx_flat = np.asarray(token_indices).reshape(-1).astype(np.int64)

    plan = _make_plan(idx_flat, n_tokens, N_CORES)
    in_maps = [_pack_core_inputs(plan, m, x_flat, w_flat, D)
               for m in range(N_CORES)]
    nc = _build_program(plan, D, N_CORES)

    res = bass_utils.run_bass_kernel_spmd(
        nc, in_maps, core_ids=list(range(N_CORES)), trace=False,
    )
    n_win = plan["n_win"]
    out = np.zeros((n_tokens, D), np.float32)
    for m in range(N_CORES):
        t_ne = plan["tok_ne"][m]
        o = np.asarray(res.results[m]["out"]).reshape(P, n_win, D)
        o = o.transpose(1, 0, 2).reshape(n_win * P, D)
        out[t_ne] = o[:len(t_ne)].astype(np.float32)
    return out.reshape(B, S, D)


# revision 27
# speedup vs baseline: 1.0879x; 1.0879x over previous
"""MoE ExpertCombiner (scatter-add) Trainium2 Bass kernel.

  out[b, s, :] = sum over (e, c) with token_indices[e,c] == b*S+s of
                 weights[e, c] * expert_outputs[e, c, :]

Strategy (8 NeuronCores, SPMD):
  Host: flatten the (e, c) rows, stable-sort by destination token, and
  shard the TOKEN space contiguously across the 8 cores (each core owns
  4096 destination tokens and receives exactly the sorted rows that land
  in its range -> no cross-core reduction at all; outputs concatenate).

  Device: the scatter-add becomes block-diagonal one-hot matmuls.  For
  each 128-token output window, PSUM accumulates
      onehot[rows_chunk, 128].T @ x[rows_chunk, D]
  over the few 128-row chunks of the sorted stream that overlap the
  window.  The per-row combine weight is folded into the one-hot on
  VectorE ((iota == idx) * w in a single tensor_scalar), so the PE does
  the weighting for free and no per-element multiply over D is needed.

  Everything bulky moves as bf16: rows are pre-cast on the host (the
  2e-2 harness tolerance leaves ~8x headroom over bf16's ~2.5e-3), and
  the f32 PSUM result is cast to bf16 on the PSUM->SBUF copy.  The
  core-local output lives in DRAM as [128, n_win*D] (partition-major)
  so completed windows drain as a few 2MB DMAs; the host undoes the
  transpose when assembling the full [B,S,D] f32 output.

Per-core traffic is ~17MB in + 8MB out, close to the 358 GB/s per-core
HBM roofline for this op.
"""

import math

import numpy as np
import ml_dtypes

import concourse.bacc as bacc
import concourse.mybir as mybir
import concourse.tile as tile
from concourse import bass_utils

P = 128
F32 = mybir.dt.float32
BF16 = mybir.dt.bfloat16
NP_BF16 = ml_dtypes.bfloat16

N_CORES = 8
W_TOK = 128


def _make_plan(idx_flat, n_tokens, n_cores, w_tok=128, group_chunks=8):
    """Sort/shard/window planning. Returns plan dict (shared across cores).

    Empty-token skip: each core's owned tokens are rank-compacted to the
    non-empty ones (the harness output buffer is pre-zeroed, so tokens
    that receive no rows need no device output at all).  Device windows
    cover RANK space; the host scatters ranks back to token ids.
    """
    order = np.argsort(idx_flat, kind="stable")
    idx_s = idx_flat[order]
    tok_per_core = n_tokens // n_cores
    bounds = np.searchsorted(idx_s, np.arange(n_cores + 1) * tok_per_core)
    counts = np.diff(bounds)
    R = int(counts.max())
    nchunk = math.ceil(R / P)
    ngrp = math.ceil(nchunk / group_chunks)
    nchunk_pad = ngrp * group_chunks
    npad = nchunk_pad * P

    cnt = np.bincount(idx_flat, minlength=n_tokens)
    tok_ne = []          # per core: global token ids of its non-empty tokens
    ranks = []           # per core: rank of each of its (sorted) rows
    for m in range(n_cores):
        lo = m * tok_per_core
        t_ne = np.flatnonzero(cnt[lo:lo + tok_per_core] > 0)
        rank_of = np.full(tok_per_core, -1, np.int64)
        rank_of[t_ne] = np.arange(len(t_ne))
        tok_ne.append(t_ne + lo)
        ranks.append(rank_of[idx_s[bounds[m]:bounds[m + 1]] - lo])
    n_win = max(math.ceil(len(t) / w_tok) for t in tok_ne)

    c_lo = np.full(n_win, 1 << 30, np.int64)
    c_hi = np.full(n_win, -1, np.int64)
    for m in range(n_cores):
        il = ranks[m]
        ws = np.searchsorted(il, np.arange(n_win + 1) * w_tok)
        s_, e_ = ws[:-1], ws[1:]
        ne = e_ > s_
        # non-empty windows: chunks containing their first/last row
        c_lo[ne] = np.minimum(c_lo[ne], s_[ne] // P)
        c_hi[ne] = np.maximum(c_hi[ne], (e_[ne] - 1) // P)
        # empty windows: point at the adjacent chunk (its indices fall
        # outside the window, so the one-hot is all-zero and the matmul
        # just zeroes PSUM) without distorting any chunk's window span
        emp = ~ne
        adj = np.minimum(s_[emp] // P, nchunk - 1)
        c_lo[emp] = np.minimum(c_lo[emp], adj)
        c_hi[emp] = np.maximum(c_hi[emp], adj)
    c_lo = np.clip(c_lo, 0, nchunk - 1)
    c_hi = np.clip(c_hi, 0, nchunk - 1)
    c_hi = np.maximum(c_hi, c_lo)

    pairs = []
    win_pair_slices = []
    for w in range(n_win):
        s = len(pairs)
        for c in range(int(c_lo[w]), int(c_hi[w]) + 1):
            pairs.append((w, c))
        win_pair_slices.append((s, len(pairs)))

    chunk_wfirst = {}
    chunk_span = {}
    for w, c in pairs:
        if c not in chunk_wfirst:
            chunk_wfirst[c] = w
        chunk_span[c] = w - chunk_wfirst[c] + 1
    w_span = max(chunk_span.values()) if chunk_span else 1

    return dict(
        order=order, idx_s=idx_s, bounds=bounds, n_win=n_win, w_tok=w_tok,
        tok_per_core=tok_per_core, nchunk=nchunk, nchunk_pad=nchunk_pad,
        npad=npad, ngrp=ngrp, pairs=pairs,
        win_pair_slices=win_pair_slices, n_cores=n_cores,
        group_chunks=group_chunks, chunk_wfirst=chunk_wfirst,
        chunk_span=chunk_span, w_span=w_span,
        tok_ne=tok_ne, ranks=ranks,
    )


def _pack_core_inputs(plan, m, x_flat, w_flat, D):
    """Build in_map arrays for core m.

    rows: bf16 [P, nchunk_pad*D], layout [p, (g, k, d)] so that group g's
      chunks are contiguous columns (one big DMA per group).
    meta: [128, nchunk*2] f32
      cols [0, nchunk)          : per-chunk weight column
      cols [nchunk, 2*nchunk)   : per-chunk window-relative index column
    """
    order, idx_s, bounds = plan["order"], plan["idx_s"], plan["bounds"]
    npad, nchunk = plan["npad"], plan["nchunk"]
    w_tok, tok_per_core = plan["w_tok"], plan["tok_per_core"]
    gch, ngrp = plan["group_chunks"], plan["ngrp"]
    sel = order[bounds[m]:bounds[m + 1]]
    Rm = len(sel)
    rows = np.zeros((npad, D), NP_BF16)
    rows[:Rm] = x_flat[sel].astype(NP_BF16)
    rows = np.ascontiguousarray(
        rows.reshape(ngrp, gch, P, D).transpose(2, 0, 1, 3)
    ).reshape(P, ngrp * gch * D)

    wv = np.zeros(nchunk * P, np.float32)
    wv[:Rm] = w_flat[sel]
    il = np.full(nchunk * P, -(1 << 20), np.float32)
    il[:Rm] = plan["ranks"][m].astype(np.float32)

    meta = np.zeros((P, nchunk * 2), np.float32)
    meta[:, :nchunk] = wv.reshape(nchunk, P).T
    ilm = il.reshape(nchunk, P).T.copy()
    for c, wf in plan["chunk_wfirst"].items():
        ilm[:, c] -= wf * w_tok
    meta[:, nchunk:] = ilm

    wide = plan["w_span"] * w_tok
    iota = np.broadcast_to(np.arange(wide, dtype=np.float32), (P, wide)).copy()
    return {"rows": rows, "meta": meta, "iota": iota}


def _build_program(plan, D, n_cores, group_bufs=5, stage_bufs=3,
                   psum_bufs=4, onehot_bufs=12, kb=4, vec_cols=160,
                   oh_ahead=2):
    n_win, w_tok = plan["n_win"], plan["w_tok"]
    nchunk, nchunk_pad = plan["nchunk"], plan["nchunk_pad"]
    pairs, win_pair_slices = plan["pairs"], plan["win_pair_slices"]
    gch, ngrp = plan["group_chunks"], plan["ngrp"]
    chunk_wfirst = plan["chunk_wfirst"]
    chunk_span = plan["chunk_span"]
    w_span = plan["w_span"]
    half = min(D, 512)
    n_half = D // half
    eq = mybir.AluOpType.is_equal

    nc = bacc.Bacc("TRN2", target_bir_lowering=False, debug=False,
                   enable_asserts=False, num_devices=n_cores)
    rows_d = nc.dram_tensor("rows", [P, nchunk_pad * D], BF16,
                            kind="ExternalInput").ap()
    meta_d = nc.dram_tensor("meta", [P, nchunk * 2], F32,
                            kind="ExternalInput").ap()
    iota_d = nc.dram_tensor("iota", [P, w_span * w_tok], F32,
                            kind="ExternalInput").ap()
    out_d = nc.dram_tensor("out", [P, n_win * D], BF16,
                           kind="ExternalOutput").ap()

    with tile.TileContext(nc) as tc:
        with (
            tc.tile_pool(name="grp", bufs=group_bufs) as gpool,
            tc.tile_pool(name="misc", bufs=1) as mpool,
            tc.tile_pool(name="stage", bufs=stage_bufs) as spool,
            tc.tile_pool(name="oh", bufs=onehot_bufs) as opool,
            tc.tile_pool(name="ps", bufs=psum_bufs, space="PSUM") as ppool,
        ):
            # iota/meta lead the sync (HWDGE) queue: they're tiny (~0.26MB)
            # and the first one-hot build needs them; the SWDGE/gpsimd
            # alternative costs ~3.5us in Q7 descriptor drain
            iota_t = mpool.tile([P, w_span * w_tok], F32)
            nc.sync.dma_start(out=iota_t[:], in_=iota_d[:])
            meta_t = mpool.tile([P, nchunk * 2], F32)
            nc.sync.dma_start(out=meta_t[:], in_=meta_d[:])

            group_tiles = {}
            oh_tiles = {}

            def get_group(g):
                t = group_tiles.get(g)
                if t is None:
                    t = gpool.tile([P, gch * D], BF16, tag="grp")
                    base = g * gch * D
                    ncol = (min(nchunk, (g + 1) * gch) - g * gch) * D
                    if g < 2:
                        # per-chunk sub-DMAs so early matmuls depend on
                        # individual chunks, not the whole 2MB group; the
                        # very first chunk lands in two half-chunk pieces,
                        # then iota/meta slot in right behind it
                        for j in range(ncol // D):
                            sub = 2 if (g == 0 and j == 0) else 1
                            for q in range(sub):
                                c0 = j * D + q * D // sub
                                c1 = j * D + (q + 1) * D // sub
                                nc.sync.dma_start(
                                    out=t[:, c0:c1],
                                    in_=rows_d[:, base + c0:base + c1],
                                )
                    else:
                        nc.sync.dma_start(
                            out=t[:, :ncol],
                            in_=rows_d[:, base:base + ncol],
                        )
                    group_tiles[g] = t
                return t

            def get_oh(c):
                """One-hot (scaled by the combine weight) for chunk c over
                its window span: oh[p, t] = (il[p] == t) * w[p]."""
                t = oh_tiles.get(c)
                if t is None:
                    ncols = chunk_span.get(c, 1) * w_tok
                    t = opool.tile([P, w_span * w_tok], BF16, tag="oh")
                    nc.vector.tensor_scalar(
                        t[:, :ncols], iota_t[:, :ncols],
                        meta_t[:, nchunk + c:nchunk + c + 1],
                        meta_t[:, c:c + 1],
                        op0=eq, op1=mybir.AluOpType.mult,
                    )
                    oh_tiles[c] = t
                return t

            st = None
            for w in range(n_win):
                # prefetch one-hot builds a couple of windows ahead so the
                # PE never waits on VectorE mid-stream
                for ww in range(w, min(w + 1 + oh_ahead, n_win)):
                    ss, ee = win_pair_slices[ww]
                    for j in range(ss, ee):
                        get_oh(pairs[j][1])
                ps = ppool.tile([P, D], F32)
                s, e = win_pair_slices[w]
                for j in range(s, e):
                    _, c = pairs[j]
                    first, last = (j == s), (j == e - 1)
                    oh = get_oh(c)
                    g, k = divmod(c, gch)
                    gt = get_group(g)
                    off = (w - chunk_wfirst[c]) * w_tok
                    ohs = oh[:, off:off + w_tok]
                    for h in range(n_half):
                        hs = slice(h * half, (h + 1) * half)
                        nc.tensor.matmul(
                            ps[:, hs], ohs,
                            gt[:, k * D + h * half:k * D + (h + 1) * half],
                            start=first, stop=last,
                        )
                if w % kb == 0:
                    st = spool.tile([P, kb * D], BF16, tag="st")
                base = (w % kb) * D
                vc = D // 2 if w == n_win - 1 else vec_cols
                nc.vector.tensor_copy(st[:, base:base + vc],
                                      ps[:, :vc])
                nc.scalar.activation(st[:, base + vc:base + D],
                                     ps[:, vc:],
                                     mybir.ActivationFunctionType.Copy)
                last_stage = w >= (n_win - 1) // kb * kb
                if last_stage:
                    # tail: drain window-by-window so the final DMA is small
                    nc.scalar.dma_start(
                        out=out_d[:, w * D:(w + 1) * D],
                        in_=st[:, base:base + D],
                    )
                elif w % kb == kb - 1:
                    w0 = (w // kb) * kb
                    nc.scalar.dma_start(
                        out=out_d[:, w0 * D:(w + 1) * D],
                        in_=st[:, :kb * D],
                    )

    nc.compile()
    return nc


def kernel(expert_outputs, weights, token_indices, batch_size, seq_len):
    expert_outputs = np.ascontiguousarray(expert_outputs, dtype=np.float32)
    weights = np.ascontiguousarray(weights, dtype=np.float32)
    B, S = int(batch_size), int(seq_len)
    E, C, D = expert_outputs.shape
    n_tokens = B * S

    x_flat = expert_outputs.reshape(-1, D)
    w_flat = weights.reshape(-1)
    idx_flat = np.asarray(token_indices).reshape(-1).astype(np.int64)

    plan = _make_plan(idx_flat, n_tokens, N_CORES)
    in_maps = [_pack_core_inputs(plan, m, x_flat, w_flat, D)
               for m in range(N_CORES)]
    nc = _build_program(plan, D, N_CORES)

    res = bass_utils.run_bass_kernel_spmd(
        nc, in_maps, core_ids=list(range(N_CORES)), trace=False,
    )
    n_win = plan["n_win"]
    out = np.zeros((n_tokens, D), np.float32)
    for m in range(N_CORES):
        t_ne = plan["tok_ne"][m]
        o = np.asarray(res.results[m]["out"]).reshape(P, n_win, D)
        o = o.transpose(1, 0, 2).reshape(n_win * P, D)
        out[t_ne] = o[:len(t_ne)].astype(np.float32)
    return out.reshape(B, S, D)


# revision 32
# speedup vs baseline: 1.1091x; 1.0194x over previous
"""MoE ExpertCombiner (scatter-add) Trainium2 Bass kernel.

  out[b, s, :] = sum over (e, c) with token_indices[e,c] == b*S+s of
                 weights[e, c] * expert_outputs[e, c, :]

Strategy (8 NeuronCores, SPMD):
  Host: flatten the (e, c) rows, stable-sort by destination token, and
  shard the TOKEN space contiguously across the 8 cores (each core owns
  4096 destination tokens and receives exactly the sorted rows that land
  in its range -> no cross-core reduction at all; outputs concatenate).

  Device: the scatter-add becomes block-diagonal one-hot matmuls.  For
  each 128-token output window, PSUM accumulates
      onehot[rows_chunk, 128].T @ x[rows_chunk, D]
  over the few 128-row chunks of the sorted stream that overlap the
  window.  The per-row combine weight is folded into the one-hot on
  VectorE ((iota == idx) * w in a single tensor_scalar), so the PE does
  the weighting for free and no per-element multiply over D is needed.

  Everything bulky moves as bf16: rows are pre-cast on the host (the
  2e-2 harness tolerance leaves ~8x headroom over bf16's ~2.5e-3), and
  the f32 PSUM result is cast to bf16 on the PSUM->SBUF copy.  The
  core-local output lives in DRAM as [128, n_win*D] (partition-major)
  so completed windows drain as a few 2MB DMAs; the host undoes the
  transpose when assembling the full [B,S,D] f32 output.

Per-core traffic is ~17MB in + 8MB out, close to the 358 GB/s per-core
HBM roofline for this op.
"""

import math

import numpy as np
import ml_dtypes

import concourse.bacc as bacc
import concourse.mybir as mybir
import concourse.tile as tile
from concourse import bass_utils

P = 128
F32 = mybir.dt.float32
BF16 = mybir.dt.bfloat16
NP_BF16 = ml_dtypes.bfloat16

N_CORES = 8
W_TOK = 128


def _make_plan(idx_flat, n_tokens, n_cores, w_tok=128, group_chunks=16):
    """Sort/shard/window planning. Returns plan dict (shared across cores).

    Empty-token skip: each core's owned tokens are rank-compacted to the
    non-empty ones (the harness output buffer is pre-zeroed, so tokens
    that receive no rows need no device output at all).  Device windows
    cover RANK space; the host scatters ranks back to token ids.
    """
    order = np.argsort(idx_flat, kind="stable")
    idx_s = idx_flat[order]
    tok_per_core = n_tokens // n_cores
    bounds = np.searchsorted(idx_s, np.arange(n_cores + 1) * tok_per_core)
    counts = np.diff(bounds)
    R = int(counts.max())
    nchunk = math.ceil(R / P)
    ngrp = math.ceil(nchunk / group_chunks)
    nchunk_pad = ngrp * group_chunks
    npad = nchunk_pad * P

    cnt = np.bincount(idx_flat, minlength=n_tokens)
    tok_ne = []          # per core: global token ids of its non-empty tokens
    ranks = []           # per core: rank of each of its (sorted) rows
    for m in range(n_cores):
        lo = m * tok_per_core
        t_ne = np.flatnonzero(cnt[lo:lo + tok_per_core] > 0)
        rank_of = np.full(tok_per_core, -1, np.int64)
        rank_of[t_ne] = np.arange(len(t_ne))
        tok_ne.append(t_ne + lo)
        ranks.append(rank_of[idx_s[bounds[m]:bounds[m + 1]] - lo])
    n_win = max(math.ceil(len(t) / w_tok) for t in tok_ne)

    c_lo = np.full(n_win, 1 << 30, np.int64)
    c_hi = np.full(n_win, -1, np.int64)
    for m in range(n_cores):
        il = ranks[m]
        ws = np.searchsorted(il, np.arange(n_win + 1) * w_tok)
        s_, e_ = ws[:-1], ws[1:]
        ne = e_ > s_
        # non-empty windows: chunks containing their first/last row
        c_lo[ne] = np.minimum(c_lo[ne], s_[ne] // P)
        c_hi[ne] = np.maximum(c_hi[ne], (e_[ne] - 1) // P)
        # empty windows: point at the adjacent chunk (its indices fall
        # outside the window, so the one-hot is all-zero and the matmul
        # just zeroes PSUM) without distorting any chunk's window span
        emp = ~ne
        adj = np.minimum(s_[emp] // P, nchunk - 1)
        c_lo[emp] = np.minimum(c_lo[emp], adj)
        c_hi[emp] = np.maximum(c_hi[emp], adj)
    c_lo = np.clip(c_lo, 0, nchunk - 1)
    c_hi = np.clip(c_hi, 0, nchunk - 1)
    c_hi = np.maximum(c_hi, c_lo)

    pairs = []
    win_pair_slices = []
    for w in range(n_win):
        s = len(pairs)
        for c in range(int(c_lo[w]), int(c_hi[w]) + 1):
            pairs.append((w, c))
        win_pair_slices.append((s, len(pairs)))

    chunk_wfirst = {}
    chunk_span = {}
    for w, c in pairs:
        if c not in chunk_wfirst:
            chunk_wfirst[c] = w
        chunk_span[c] = w - chunk_wfirst[c] + 1
    w_span = max(chunk_span.values()) if chunk_span else 1

    return dict(
        order=order, idx_s=idx_s, bounds=bounds, n_win=n_win, w_tok=w_tok,
        tok_per_core=tok_per_core, nchunk=nchunk, nchunk_pad=nchunk_pad,
        npad=npad, ngrp=ngrp, pairs=pairs,
        win_pair_slices=win_pair_slices, n_cores=n_cores,
        group_chunks=group_chunks, chunk_wfirst=chunk_wfirst,
        chunk_span=chunk_span, w_span=w_span,
        tok_ne=tok_ne, ranks=ranks,
    )


def _pack_core_inputs(plan, m, x_flat, w_flat, D):
    """Build in_map arrays for core m.

    rows: bf16 [P, nchunk_pad*D], layout [p, (g, k, d)] so that group g's
      chunks are contiguous columns (one big DMA per group).
    meta: [128, nchunk*2] f32
      cols [0, nchunk)          : per-chunk weight column
      cols [nchunk, 2*nchunk)   : per-chunk window-relative index column
    """
    order, idx_s, bounds = plan["order"], plan["idx_s"], plan["bounds"]
    npad, nchunk = plan["npad"], plan["nchunk"]
    w_tok, tok_per_core = plan["w_tok"], plan["tok_per_core"]
    gch, ngrp = plan["group_chunks"], plan["ngrp"]
    sel = order[bounds[m]:bounds[m + 1]]
    Rm = len(sel)
    rows = np.zeros((npad, D), NP_BF16)
    rows[:Rm] = x_flat[sel].astype(NP_BF16)
    rows = np.ascontiguousarray(
        rows.reshape(ngrp, gch, P, D).transpose(2, 0, 1, 3)
    ).reshape(P, ngrp * gch * D)

    wv = np.zeros(nchunk * P, np.float32)
    wv[:Rm] = w_flat[sel]
    il = np.full(nchunk * P, -(1 << 20), np.float32)
    il[:Rm] = plan["ranks"][m].astype(np.float32)

    meta = np.zeros((P, nchunk * 2), np.float32)
    meta[:, :nchunk] = wv.reshape(nchunk, P).T
    ilm = il.reshape(nchunk, P).T.copy()
    for c, wf in plan["chunk_wfirst"].items():
        ilm[:, c] -= wf * w_tok
    meta[:, nchunk:] = ilm

    wide = plan["w_span"] * w_tok
    iota = np.broadcast_to(np.arange(wide, dtype=np.float32), (P, wide)).copy()
    return {"rows": rows, "meta": meta, "iota": iota}


def _build_program(plan, D, n_cores, group_bufs=3, stage_bufs=2,
                   psum_bufs=4, onehot_bufs=12, kb=8, vec_cols=160,
                   oh_ahead=2):
    n_win, w_tok = plan["n_win"], plan["w_tok"]
    nchunk, nchunk_pad = plan["nchunk"], plan["nchunk_pad"]
    pairs, win_pair_slices = plan["pairs"], plan["win_pair_slices"]
    gch, ngrp = plan["group_chunks"], plan["ngrp"]
    chunk_wfirst = plan["chunk_wfirst"]
    chunk_span = plan["chunk_span"]
    w_span = plan["w_span"]
    half = min(D, 512)
    n_half = D // half
    eq = mybir.AluOpType.is_equal

    nc = bacc.Bacc("TRN2", target_bir_lowering=False, debug=False,
                   enable_asserts=False, num_devices=n_cores)
    rows_d = nc.dram_tensor("rows", [P, nchunk_pad * D], BF16,
                            kind="ExternalInput").ap()
    meta_d = nc.dram_tensor("meta", [P, nchunk * 2], F32,
                            kind="ExternalInput").ap()
    iota_d = nc.dram_tensor("iota", [P, w_span * w_tok], F32,
                            kind="ExternalInput").ap()
    out_d = nc.dram_tensor("out", [P, n_win * D], BF16,
                           kind="ExternalOutput").ap()

    with tile.TileContext(nc) as tc:
        with (
            tc.tile_pool(name="grp", bufs=group_bufs) as gpool,
            tc.tile_pool(name="misc", bufs=1) as mpool,
            tc.tile_pool(name="stage", bufs=stage_bufs) as spool,
            tc.tile_pool(name="oh", bufs=onehot_bufs) as opool,
            tc.tile_pool(name="ps", bufs=psum_bufs, space="PSUM") as ppool,
        ):
            # iota/meta ride the gpsimd (SWDGE) queue so the sync queue
            # carries nothing but row data — inserting them ahead of the
            # rows delays every chunk arrival and measures ~3us slower
            iota_t = mpool.tile([P, w_span * w_tok], F32)
            nc.gpsimd.dma_start(out=iota_t[:], in_=iota_d[:])
            meta_t = mpool.tile([P, nchunk * 2], F32)
            nc.gpsimd.dma_start(out=meta_t[:], in_=meta_d[:])

            group_tiles = {}
            oh_tiles = {}

            def get_group(g):
                t = group_tiles.get(g)
                if t is None:
                    t = gpool.tile([P, gch * D], BF16, tag="grp")
                    base = g * gch * D
                    ncol = (min(nchunk, (g + 1) * gch) - g * gch) * D
                    if g == 0:
                        # fine-grained leading DMAs so early matmuls depend
                        # on individual chunks: chunk 0 in two halves,
                        # chunks 1-7 singly, the rest of the group in one
                        for j in range(min(8, ncol // D)):
                            sub = 2 if j == 0 else 1
                            for q in range(sub):
                                c0 = j * D + q * D // sub
                                c1 = j * D + (q + 1) * D // sub
                                nc.sync.dma_start(
                                    out=t[:, c0:c1],
                                    in_=rows_d[:, base + c0:base + c1],
                                )
                        if ncol > 8 * D:
                            nc.sync.dma_start(
                                out=t[:, 8 * D:ncol],
                                in_=rows_d[:, base + 8 * D:base + ncol],
                            )
                    elif g == 1 and ncol > gch * D // 2:
                        hcol = gch * D // 2
                        nc.sync.dma_start(out=t[:, :hcol],
                                          in_=rows_d[:, base:base + hcol])
                        nc.sync.dma_start(out=t[:, hcol:ncol],
                                          in_=rows_d[:, base + hcol:base + ncol])
                    else:
                        nc.sync.dma_start(
                            out=t[:, :ncol],
                            in_=rows_d[:, base:base + ncol],
                        )
                    group_tiles[g] = t
                return t

            def get_oh(c):
                """One-hot (scaled by the combine weight) for chunk c over
                its window span: oh[p, t] = (il[p] == t) * w[p]."""
                t = oh_tiles.get(c)
                if t is None:
                    ncols = chunk_span.get(c, 1) * w_tok
                    t = opool.tile([P, w_span * w_tok], BF16, tag="oh")
                    nc.vector.tensor_scalar(
                        t[:, :ncols], iota_t[:, :ncols],
                        meta_t[:, nchunk + c:nchunk + c + 1],
                        meta_t[:, c:c + 1],
                        op0=eq, op1=mybir.AluOpType.mult,
                    )
                    oh_tiles[c] = t
                return t

            st = None
            for w in range(n_win):
                # prefetch one-hot builds a couple of windows ahead so the
                # PE never waits on VectorE mid-stream
                for ww in range(w, min(w + 1 + oh_ahead, n_win)):
                    ss, ee = win_pair_slices[ww]
                    for j in range(ss, ee):
                        get_oh(pairs[j][1])
                ps = ppool.tile([P, D], F32)
                s, e = win_pair_slices[w]
                for j in range(s, e):
                    _, c = pairs[j]
                    first, last = (j == s), (j == e - 1)
                    oh = get_oh(c)
                    g, k = divmod(c, gch)
                    gt = get_group(g)
                    off = (w - chunk_wfirst[c]) * w_tok
                    ohs = oh[:, off:off + w_tok]
                    for h in range(n_half):
                        hs = slice(h * half, (h + 1) * half)
                        nc.tensor.matmul(
                            ps[:, hs], ohs,
                            gt[:, k * D + h * half:k * D + (h + 1) * half],
                            start=first, stop=last,
                        )
                if w % kb == 0:
                    st = spool.tile([P, kb * D], BF16, tag="st")
                base = (w % kb) * D
                vc = D // 2 if w == n_win - 1 else vec_cols
                nc.vector.tensor_copy(st[:, base:base + vc],
                                      ps[:, :vc])
                nc.scalar.activation(st[:, base + vc:base + D],
                                     ps[:, vc:],
                                     mybir.ActivationFunctionType.Copy)
                last_stage = w >= (n_win - 1) // kb * kb
                if last_stage:
                    # tail: drain window-by-window so the final DMA is small
                    nc.scalar.dma_start(
                        out=out_d[:, w * D:(w + 1) * D],
                        in_=st[:, base:base + D],
                    )
                elif w % kb == kb - 1:
                    w0 = (w // kb) * kb
                    nc.scalar.dma_start(
                        out=out_d[:, w0 * D:(w + 1) * D],
                        in_=st[:, :kb * D],
                    )

    nc.compile()
    return nc


def kernel(expert_outputs, weights, token_indices, batch_size, seq_len):
    expert_outputs = np.ascontiguousarray(expert_outputs, dtype=np.float32)
    weights = np.ascontiguousarray(weights, dtype=np.float32)
    B, S = int(batch_size), int(seq_len)
    E, C, D = expert_outputs.shape
    n_tokens = B * S

    x_flat = expert_outputs.reshape(-1, D)
    w_flat = weights.reshape(-1)
    idx_flat = np.asarray(token_indices).reshape(-1).astype(np.int64)

    plan = _make_plan(idx_flat, n_tokens, N_CORES)
    in_maps = [_pack_core_inputs(plan, m, x_flat, w_flat, D)
               for m in range(N_CORES)]
    nc = _build_program(plan, D, N_CORES)

    res = bass_utils.run_bass_kernel_spmd(
        nc, in_maps, core_ids=list(range(N_CORES)), trace=False,
    )
    n_win = plan["n_win"]
    out = np.zeros((n_tokens, D), np.float32)
    for m in range(N_CORES):
        t_ne = plan["tok_ne"][m]
        o = np.asarray(res.results[m]["out"]).reshape(P, n_win, D)
        o = o.transpose(1, 0, 2).reshape(n_win * P, D)
        out[t_ne] = o[:len(t_ne)].astype(np.float32)
    return out.reshape(B, S, D)
